# revision 2
# baseline (speedup 1.0000x reference)
"""GNN NodeModel kernel for 8 Trainium2 NeuronCores (Bass/Tile) — fused V3.

Design (per core, nodes sharded 2500/core, edges sorted by destination):
  Phase 1 (xw): dedup sources — compute xw = x[u] @ W1a_top + b1a for the
    core's ~8000 unique source nodes (host supplies x[u] pre-transposed in
    bf16, so the matmul needs no on-chip transposes), write bf16 rows to a
    DRAM scratch.
  Phase 2 (fused edge+segment): nodes are packed into 20 tiles of <=128
    nodes; each tile owns G_q 128-edge groups.  Per group:
      - gather xw rows for the group's edges (indirect DMA, bf16)
      - h = xw_gathered (identity matmul) + ea @ W1a_bot   [edge-major PSUM]
        (edge_attr comes in host-transposed bf16 slot order -> direct loads,
         used as the stationary operand; no gathers, no transposes)
      - g = relu(h) -> SBUF bf16
      - segment-mean accumulation, feature-major: prT[k] += g[:,k].T @ S'
        where S' is a host-built selection matrix whose entries are 1/deg
        (so the accumulated result is already the mean, already transposed
        for the next matmul; empty nodes give 0).
  Phase 3 (mm2, interleaved per 2 tiles): standard folded node MLP2:
      out = relu(x@B1 + mean@(W1b@B2) + (b1b@B2) x nonempty + b2a) @ W2b + b2b
    with [B1;B2] = W2a; x arrives host-transposed bf16; output is written
    transposed in bf16 and unpacked/cast on the host.

Everything on the PE runs in bf16 (f32 PSUM accumulation).
"""

import sys

sys.path.insert(0, "/opt/trn_rl_repo")

from contextlib import ExitStack

import numpy as np
import ml_dtypes

import concourse.bass as bass
import concourse.tile as tile
from concourse import bacc, mybir
from concourse.bass_utils import run_bass_kernel_spmd

N = 20000
E = 80000
D = 1024
C = 8
NPC = N // C      # 2500 nodes per core
NT = 20           # node tiles per core
NP = NT * 128     # 2560 padded node slots
F32 = mybir.dt.float32
BF16 = mybir.dt.bfloat16
I32 = mybir.dt.int32
BF = ml_dtypes.bfloat16

AF = mybir.ActivationFunctionType

_PROGRAM_CACHE = {}
DEFAULT_PROGRAM_ARGS = (80, 63, (4,) * 20)


def _build_program(NG, GU, pattern):
    """NG: total 128-edge groups/core (sum of pattern); GU: 128-row groups of
    unique source nodes; pattern: groups per node tile (len NT)."""
    assert sum(pattern) == NG and len(pattern) == NT
    UP = GU * 128
    S = NG * 128
    KC = D // 128  # 8

    nc = bacc.Bacc("TRN2", target_bir_lowering=False, debug=False, num_devices=C)

    # ---- DRAM I/O ----
    xtu_d = nc.dram_tensor("xtu_d", [D, UP], BF16, kind="ExternalInput").ap()
    eat_d = nc.dram_tensor("eat_d", [D, S], BF16, kind="ExternalInput").ap()
    sp_d = nc.dram_tensor("sp_d", [128, S], BF16, kind="ExternalInput").ap()
    srcidx = nc.dram_tensor("srcidx", [128, NG], I32, kind="ExternalInput").ap()
    x_myT = nc.dram_tensor("x_myT", [D, NP], BF16, kind="ExternalInput").ap()
    maskv = nc.dram_tensor("maskv", [1, NP], BF16, kind="ExternalInput").ap()
    ones_d = nc.dram_tensor("ones_d", [1, 128], BF16, kind="ExternalInput").ap()
    w1a_d = nc.dram_tensor("w1a_d", [2 * D, D], BF16, kind="ExternalInput").ap()
    wb1_d = nc.dram_tensor("wb1_d", [D, D], BF16, kind="ExternalInput").ap()
    w3_d = nc.dram_tensor("w3_d", [D, D], BF16, kind="ExternalInput").ap()
    w2b_d = nc.dram_tensor("w2b_d", [D, D], BF16, kind="ExternalInput").ap()
    b1a_d = nc.dram_tensor("b1a_d", [1, D], BF16, kind="ExternalInput").ap()
    u_d = nc.dram_tensor("u_d", [1, D], BF16, kind="ExternalInput").ap()
    b2a_d = nc.dram_tensor("b2a_d", [128, 8], F32, kind="ExternalInput").ap()
    b2b_d = nc.dram_tensor("b2b_d", [128, 8], F32, kind="ExternalInput").ap()
    out_myT = nc.dram_tensor("out_myT", [D, NP], BF16, kind="ExternalOutput").ap()
    xw_dram = nc.dram_tensor("xw_scratch", [UP, D], BF16).ap()

    NSB_U = (GU + 3) // 4   # xTu superblocks of 512 cols
    NSB_E = (NG + 3) // 4   # eaT/sp superblocks of 512 cols

    with tile.TileContext(nc) as tc, ExitStack() as ctx:
        cpool = ctx.enter_context(tc.tile_pool(name="consts", bufs=1))
        wpool = ctx.enter_context(tc.tile_pool(name="weights", bufs=1))
        sxu = ctx.enter_context(tc.tile_pool(name="sxu", bufs=3))
        sea = ctx.enter_context(tc.tile_pool(name="sea", bufs=2))
        ssp = ctx.enter_context(tc.tile_pool(name="ssp", bufs=2))
        pgat = ctx.enter_context(tc.tile_pool(name="pgat", bufs=6))
        pg = ctx.enter_context(tc.tile_pool(name="pg", bufs=3))
        pxw = ctx.enter_context(tc.tile_pool(name="pxw", bufs=3))
        prm = ctx.enter_context(tc.tile_pool(name="prm", bufs=2))
        pxt = ctx.enter_context(tc.tile_pool(name="pxt", bufs=2))
        po1 = ctx.enter_context(tc.tile_pool(name="po1", bufs=2))
        psBig = ctx.enter_context(tc.tile_pool(name="psBig", bufs=2, space="PSUM"))
        psPr = ctx.enter_context(tc.tile_pool(name="psPr", bufs=1, space="PSUM"))
        psMm2 = ctx.enter_context(tc.tile_pool(name="psMm2", bufs=2, space="PSUM"))

        # ---- phase-1-critical loads first (PE can start after ~3 MB) ----
        ones_sb = cpool.tile([1, 128], BF16, tag="ones")
        nc.sync.dma_start(ones_sb[:], ones_d[:])
        b1a_sb = cpool.tile([1, D], BF16, tag="b1a")
        nc.sync.dma_start(b1a_sb[:], b1a_d[:])
        w1a_sb = []
        xu = []
        hi0 = min(512, UP)
        for k in range(KC):
            tx = sxu.tile([128, 512], BF16, tag=f"xu{k}", name=f"xu0_{k}")
            nc.sync.dma_start(tx[:, :hi0], xtu_d[128 * k : 128 * (k + 1), :hi0])
            xu.append(tx)
            t = wpool.tile([128, D], BF16, tag=f"w1a{k}")
            nc.sync.dma_start(t[:], w1a_d[128 * k : 128 * (k + 1), :])
            w1a_sb.append(t)

        # ================= Phase 1: xw = x_u @ W1a_top + b1a =================
        for ug in range(GU):
            sb, col = ug // 4, ug % 4
            if col == 0 and sb > 0:
                xu = []
                hi = min(512, UP - 512 * sb)
                for k in range(KC):
                    t = sxu.tile([128, 512], BF16, tag=f"xu{k}", name=f"xu{sb}_{k}")
                    nc.sync.dma_start(
                        t[:, :hi],
                        xtu_d[128 * k : 128 * (k + 1), 512 * sb : 512 * sb + hi],
                    )
                    xu.append(t)
            pw = psBig.tile([128, D], F32, tag="big", name=f"pw{ug}")
            for h in range(2):
                for k in range(KC):
                    nc.tensor.matmul(
                        pw[:, 512 * h : 512 * (h + 1)],
                        xu[k][:, 128 * col : 128 * (col + 1)],
                        w1a_sb[k][:, 512 * h : 512 * (h + 1)],
                        start=(k == 0),
                        stop=False,
                    )
                nc.tensor.matmul(
                    pw[:, 512 * h : 512 * (h + 1)],
                    ones_sb[:],
                    b1a_sb[0:1, 512 * h : 512 * (h + 1)],
                    start=False,
                    stop=True,
                )
            xw_sb = pxw.tile([128, D], BF16, tag="xw", name=f"xwsb{ug}")
            nc.vector.tensor_copy(xw_sb[:], pw[:])
            nc.sync.dma_start(xw_dram[128 * ug : 128 * (ug + 1), :], xw_sb[:])

        # ---- remaining constants + phase 2/3 weights (overlap phase 1) ----
        srcidx_sb = cpool.tile([128, NG], I32, tag="srcidx")
        nc.sync.dma_start(srcidx_sb[:], srcidx[:])
        u_sb = cpool.tile([1, D], BF16, tag="u")
        nc.sync.dma_start(u_sb[:], u_d[:])
        b2a_sb = cpool.tile([128, 8], F32, tag="b2a")
        nc.sync.dma_start(b2a_sb[:], b2a_d[:])
        b2b_sb = cpool.tile([128, 8], F32, tag="b2b")
        nc.sync.dma_start(b2b_sb[:], b2b_d[:])
        for k in range(KC, 2 * KC):
            t = wpool.tile([128, D], BF16, tag=f"w1a{k}")
            nc.sync.dma_start(t[:], w1a_d[128 * k : 128 * (k + 1), :])
            w1a_sb.append(t)
        wb1_sb, w3_sb, w2b_sb = [], [], []
        for wd, lst, nm in ((wb1_d, wb1_sb, "wb"), (w3_d, w3_sb, "w3"),
                            (w2b_d, w2b_sb, "w2")):
            for k in range(KC):
                t = wpool.tile([128, D], BF16, tag=f"{nm}{k}")
                nc.sync.dma_start(t[:], wd[128 * k : 128 * (k + 1), :])
                lst.append(t)

        # ============ Phase 2: fused edge MLP1 + segment means ============
        # flattened group schedule; S-matmuls for group i are emitted after
        # group i+1's MLP1 matmuls so the relu (ACT) overlaps PE work, and the
        # identity-matmul (gathered xw) comes last so the gather has the whole
        # MLP1 window to land.
        items = []
        jj = 0
        for q in range(NT):
            for j in range(pattern[q]):
                items.append((q, j, jj))
                jj += 1

        state = {}   # jj -> (g_sb, spt, col, prt, first, last, q)
        rmt_by_t2 = {}
        prt_by_q = {}
        eat = spt = None

        def finalize(idx):
            """Emit deferred S-matmuls for group idx (+ tile epilogue)."""
            g_sb, spt_, col_, prt_, first, last, q_ = state.pop(idx)
            # prt spans 2 PSUM banks (4 chunks each); start=True clears
            # has_written for the WHOLE bank, so only the first chunk per
            # bank may set it — the others overwrite-on-cleared-bit.
            for k in range(KC):
                nc.tensor.matmul(
                    prt_[:, 128 * k : 128 * (k + 1)],
                    g_sb[:, 128 * k : 128 * (k + 1)],
                    spt_[:, 128 * col_ : 128 * (col_ + 1)],
                    start=(first and k % 4 == 0),
                    stop=last,
                )
            if last:
                t2_, half_ = q_ // 2, q_ % 2
                rmt_ = rmt_by_t2[t2_]
                for k in range(KC):
                    nc.vector.tensor_copy(
                        rmt_[k][:, 128 * half_ : 128 * (half_ + 1)],
                        prt_[:, 128 * k : 128 * (k + 1)],
                    )
                if half_ == 1:
                    mlp2(t2_, rmt_)

        xt_by_t2 = {}

        def prefetch_xt(t2):
            xt = []
            for k in range(KC):
                t = pxt.tile([128, 256], BF16, tag=f"xt{k}", name=f"xt{t2}_{k}")
                nc.sync.dma_start(
                    t[:],
                    x_myT[128 * k : 128 * (k + 1), 256 * t2 : 256 * (t2 + 1)],
                )
                xt.append(t)
            msk = pxt.tile([1, 256], BF16, tag="msk", name=f"msk{t2}")
            nc.sync.dma_start(msk[:], maskv[0:1, 256 * t2 : 256 * (t2 + 1)])
            xt_by_t2[t2] = (xt, msk)

        def mlp2(t2, rmt):
            xt, msk = xt_by_t2.pop(t2)

            o1 = []
            for m in range(KC):
                pb = psMm2.tile([128, 256], F32, tag="pb", name=f"pa{t2}_{m}")
                for k in range(KC):
                    nc.tensor.matmul(
                        pb[:], wb1_sb[k][:, 128 * m : 128 * (m + 1)], xt[k][:],
                        start=(k == 0), stop=False,
                    )
                # W3 split: the even tile's rm half (cols 0:128) was copied a
                # whole tile earlier; the odd half just landed, so do it last
                for k in range(KC):
                    nc.tensor.matmul(
                        pb[:, 0:128],
                        w3_sb[k][:, 128 * m : 128 * (m + 1)],
                        rmt[k][:, 0:128],
                        start=False, stop=False,
                    )
                nc.tensor.matmul(
                    pb[:], u_sb[0:1, 128 * m : 128 * (m + 1)], msk[:],
                    start=False, stop=False,
                )
                for k in range(KC):
                    nc.tensor.matmul(
                        pb[:, 128:256],
                        w3_sb[k][:, 128 * m : 128 * (m + 1)],
                        rmt[k][:, 128:256],
                        start=False, stop=(k == KC - 1),
                    )
                ot = po1.tile([128, 256], BF16, tag=f"o1{m}", name=f"o1{t2}_{m}")
                nc.scalar.activation(ot[:], pb[:], AF.Relu,
                                     bias=b2a_sb[:, m : m + 1])
                o1.append(ot)

            for m in range(KC):
                pb = psMm2.tile([128, 256], F32, tag="pb", name=f"pb{t2}_{m}")
                for k in range(KC):
                    nc.tensor.matmul(
                        pb[:], w2b_sb[k][:, 128 * m : 128 * (m + 1)], o1[k][:],
                        start=(k == 0), stop=(k == KC - 1),
                    )
                ot = po1.tile([128, 256], BF16, tag="o2", name=f"o2{t2}_{m}",
                              bufs=3)
                nc.scalar.activation(ot[:], pb[:], AF.Identity,
                                     bias=b2b_sb[:, m : m + 1])
                nc.sync.dma_start(
                    out_myT[128 * m : 128 * (m + 1), 256 * t2 : 256 * (t2 + 1)],
                    ot[:],
                )

        for q, j, jj in items:
            t2, half = q // 2, q % 2
            if half == 0 and j == 0:
                rmt_by_t2[t2] = [
                    prm.tile([128, 256], BF16, tag=f"rm{k}", name=f"rm{t2}_{k}")
                    for k in range(KC)
                ]
                prefetch_xt(t2)
            if j == 0:
                prt_by_q[q] = psPr.tile([128, D], F32, tag="pr", name=f"prt{q}")
            prt = prt_by_q[q]
            Gq = pattern[q]

            sb, col = jj // 4, jj % 4
            if col == 0:
                eat = []
                hi = min(512, S - 512 * sb)
                for k in range(KC):
                    t = sea.tile([128, 512], BF16, tag=f"ea{k}",
                                 name=f"ea{sb}_{k}")
                    nc.sync.dma_start(
                        t[:, :hi],
                        eat_d[128 * k : 128 * (k + 1), 512 * sb : 512 * sb + hi],
                    )
                    eat.append(t)
                spt = ssp.tile([128, 512], BF16, tag="sp", name=f"sp{sb}")
                nc.sync.dma_start(spt[:, :hi], sp_d[:, 512 * sb : 512 * sb + hi])

            xwg = pgat.tile([128, D], BF16, tag="xwg", name=f"xwg{jj}")
            nc.gpsimd.indirect_dma_start(
                out=xwg[:],
                out_offset=None,
                in_=xw_dram[:],
                in_offset=bass.IndirectOffsetOnAxis(
                    ap=srcidx_sb[:, jj : jj + 1], axis=0
                ),
            )

            ph = psBig.tile([128, D], F32, tag="big", name=f"ph{jj}")
            for h in range(2):
                for k in range(KC):
                    nc.tensor.matmul(
                        ph[:, 512 * h : 512 * (h + 1)],
                        eat[k][:, 128 * col : 128 * (col + 1)],
                        w1a_sb[KC + k][:, 512 * h : 512 * (h + 1)],
                        start=(k == 0),
                        stop=(k == KC - 1),
                    )
            if jj > 0:
                finalize(jj - 1)
            gt = pg.tile([128, D], BF16, tag="gt", name=f"gt{jj}")
            nc.vector.tensor_tensor(out=gt[:], in0=ph[:], in1=xwg[:],
                                    op=mybir.AluOpType.add)
            g_sb = pg.tile([128, D], BF16, tag="g", name=f"g{jj}")
            nc.scalar.activation(g_sb[:], gt[:], AF.Relu)
            state[jj] = (g_sb, spt, col, prt, j == 0, j == Gq - 1, q)
        finalize(NG - 1)

    nc.compile()
    return nc


def _get_program(NG, GU, pattern):
    key = (NG, GU, tuple(pattern))
    if key not in _PROGRAM_CACHE:
        _PROGRAM_CACHE[key] = _build_program(NG, GU, tuple(pattern))
    return _PROGRAM_CACHE[key]


def _pack_core(deg):
    """Pack NPC nodes (weights deg) into NT bins, <=128 nodes each,
    minimizing sum(ceil(load/128)). Returns list of (nodes, load)."""
    order = np.argsort(-deg, kind="stable")
    nodes = [[] for _ in range(NT)]
    load = np.zeros(NT, np.int64)
    cnt = np.zeros(NT, np.int64)
    for n in order:
        # LPT with node cap
        cand = [b for b in range(NT) if cnt[b] < 128]
        b = min(cand, key=lambda b: (load[b], cnt[b]))
        nodes[b].append(n)
        load[b] += deg[n]
        cnt[b] += 1
    # refinement: reduce sum(ceil(load/128)) by moving small nodes out of
    # bins that spill just over a multiple of 128
    for _ in range(200):
        ceil = -(-load // 128)
        improved = False
        spill_key = np.where(
            (load > 0) & (load % 128 != 0), (load - 1) % 128 + 1, 10**9
        )
        for a in np.argsort(spill_key):
            if load[a] == 0 or (load[a] % 128) == 0:
                continue
            spill = load[a] - 128 * (ceil[a] - 1)
            # try to move small nodes (total <= spill) from a to other bins
            small = sorted((deg[n], n) for n in nodes[a] if deg[n] > 0)
            moved = []
            need = spill
            for d, n in small:
                if d > need:
                    break
                tgt = None
                for b in range(NT):
                    if b == a or cnt[b] >= 128:
                        continue
                    if -(-(load[b] + d) // 128) == ceil[b]:
                        tgt = b
                        break
                if tgt is None:
                    continue
                nodes[a].remove(n)
                nodes[tgt].append(n)
                load[a] -= d
                load[tgt] += d
                cnt[a] -= 1
                cnt[tgt] += 1
                moved.append(n)
                need -= d
                if need <= 0:
                    break
            if need <= 0 and moved:
                improved = True
                break
        if not improved:
            break
    return [(nodes[b], int(load[b])) for b in range(NT)]


def _make_in_maps(x, edge_index, edge_attr, W1a, b1a, W1b, b1b, W2a, b2a, W2b, b2b):
    x = np.ascontiguousarray(np.asarray(x, np.float32))
    edge_attr = np.ascontiguousarray(np.asarray(edge_attr, np.float32))
    ei = np.asarray(edge_index)
    row, col = ei[0].astype(np.int64), ei[1].astype(np.int64)

    perm = np.argsort(col, kind="stable")
    col_s, row_s = col[perm], row[perm]
    core_bounds = np.searchsorted(col_s, NPC * np.arange(C + 1))
    counts = np.bincount(col, minlength=N)

    # ---- pack nodes per core; derive the shared group pattern ----
    packs, uniqs = [], []
    for c in range(C):
        deg = counts[NPC * c : NPC * (c + 1)]
        bins = _pack_core(deg)
        bins.sort(key=lambda bl: -bl[1])
        packs.append(bins)
        s0, e0 = core_bounds[c], core_bounds[c + 1]
        uniqs.append(np.unique(row_s[s0:e0]))
    pattern = tuple(
        int(max(-(-packs[c][q][1] // 128) for c in range(C))) for q in range(NT)
    )
    pattern = tuple(max(p, 1) for p in pattern)
    NG = sum(pattern)
    GU = max(1, -(-max(len(u) for u in uniqs) // 128))
    UP, S = GU * 128, NG * 128

    # ---- fold weights (float64 for accuracy) ----
    B1 = np.asarray(W2a, np.float64)[:D]
    B2 = np.asarray(W2a, np.float64)[D:]
    W3 = (np.asarray(W1b, np.float64) @ B2).astype(np.float32)
    u_vec = (np.asarray(b1b, np.float64) @ B2).astype(np.float32)

    x_bf = x.astype(BF)
    ea_bf = edge_attr.astype(BF)

    in_maps = []
    orders = []
    for c in range(C):
        s0 = core_bounds[c]
        lo = NPC * c
        bins = packs[c]
        uniq = uniqs[c]
        deg = counts[lo : lo + NPC]
        starts = np.zeros(NPC + 1, np.int64)
        np.cumsum(deg, out=starts[1:])

        src_l = np.zeros((128, NG), np.int32)     # local xw row per slot
        sp = np.zeros((128, S), BF)               # selection matrix, 1/deg vals
        ea_sel = np.full(S, -1, np.int64)         # edge_attr row per slot
        order = np.full(NP, -1, np.int64)         # packed node order

        goff = 0
        for q in range(NT):
            bnodes, load = bins[q]
            Gq = pattern[q]
            pos = 0
            for p, n in enumerate(bnodes):
                order[128 * q + p] = n
                d = int(deg[n])
                if d == 0:
                    continue
                ids = np.arange(starts[n], starts[n + 1], dtype=np.int64)
                sl = goff * 128 + pos + np.arange(d)
                gidx_, ridx_ = sl // 128, sl % 128
                src_l[ridx_, gidx_] = np.searchsorted(uniq, row_s[s0 + ids])
                sp[ridx_, gidx_ * 128 + p] = BF(1.0 / d)
                ea_sel[sl] = perm[s0 + ids]
                pos += d
            assert pos <= 128 * Gq, (c, q, pos, Gq)
            goff += Gq
        assert goff == NG

        # host-transposed, slot-ordered edge features
        eat = np.zeros((D, S), BF)
        nz = ea_sel >= 0
        eat[:, nz] = ea_bf[ea_sel[nz]].T
        # host-transposed unique-source features
        xtu = np.zeros((D, UP), BF)
        xtu[:, : len(uniq)] = x_bf[uniq].T

        ordc = np.maximum(order, 0)
        valid = order >= 0
        cnt_c = np.where(valid, deg[ordc], 0)
        mask_c = ((cnt_c > 0) & valid).astype(BF)
        x_c = np.where(valid[:, None], x_bf[lo + ordc], BF(0.0))

        in_maps.append(
            {
                "xtu_d": xtu,
                "eat_d": eat,
                "sp_d": sp,
                "srcidx": src_l,
                "x_myT": np.ascontiguousarray(x_c.T),
                "maskv": mask_c.reshape(1, NP),
                "ones_d": np.ones((1, 128), BF),
                "w1a_d": np.asarray(W1a, np.float32).astype(BF),
                "wb1_d": B1.astype(np.float32).astype(BF),
                "w3_d": W3.astype(BF),
                "w2b_d": np.asarray(W2b, np.float32).astype(BF),
                "b1a_d": np.asarray(b1a, np.float32).astype(BF).reshape(1, D),
                "u_d": u_vec.astype(BF).reshape(1, D),
                "b2a_d": np.asarray(b2a, np.float32).reshape(8, 128).T.copy(),
                "b2b_d": np.asarray(b2b, np.float32).reshape(8, 128).T.copy(),
            }
        )
        orders.append(order)
    return (NG, GU, pattern), in_maps, orders


def kernel(x, edge_index, edge_attr, W1a, b1a, W1b, b1b, W2a, b2a, W2b, b2b):
    args, in_maps, orders = _make_in_maps(
        x, edge_index, edge_attr, W1a, b1a, W1b, b1b, W2a, b2a, W2b, b2b
    )
    nc = _get_program(*args)
    res = run_bass_kernel_spmd(nc, in_maps, core_ids=list(range(C)))
    out = np.empty((N, D), np.float32)
    for c in range(C):
        o = np.asarray(res.results[c]["out_myT"]).astype(np.float32).T  # [NP, D]
        order = orders[c]
        valid = order >= 0
        out[NPC * c + order[valid]] = o[valid]
    return np.ascontiguousarray(out)


# revision 3
# speedup vs baseline: 1.0011x; 1.0011x over previous
"""GNN NodeModel kernel for 8 Trainium2 NeuronCores (Bass/Tile) — fused V3.

Design (per core, nodes sharded 2500/core, edges sorted by destination):
  Phase 1 (xw): dedup sources — compute xw = x[u] @ W1a_top + b1a for the
    core's ~8000 unique source nodes (host supplies x[u] pre-transposed in
    bf16, so the matmul needs no on-chip transposes), write bf16 rows to a
    DRAM scratch.
  Phase 2 (fused edge+segment): nodes are packed into 20 tiles of <=128
    nodes; each tile owns G_q 128-edge groups.  Per group:
      - gather xw rows for the group's edges (indirect DMA, bf16)
      - h = xw_gathered (identity matmul) + ea @ W1a_bot   [edge-major PSUM]
        (edge_attr comes in host-transposed bf16 slot order -> direct loads,
         used as the stationary operand; no gathers, no transposes)
      - g = relu(h) -> SBUF bf16
      - segment-mean accumulation, feature-major: prT[k] += g[:,k].T @ S'
        where S' is a host-built selection matrix whose entries are 1/deg
        (so the accumulated result is already the mean, already transposed
        for the next matmul; empty nodes give 0).
  Phase 3 (mm2, interleaved per 2 tiles): standard folded node MLP2:
      out = relu(x@B1 + mean@(W1b@B2) + (b1b@B2) x nonempty + b2a) @ W2b + b2b
    with [B1;B2] = W2a; x arrives host-transposed bf16; output is written
    transposed in bf16 and unpacked/cast on the host.

Everything on the PE runs in bf16 (f32 PSUM accumulation).
"""

import sys

sys.path.insert(0, "/opt/trn_rl_repo")

from contextlib import ExitStack

import numpy as np
import ml_dtypes

import concourse.bass as bass
import concourse.tile as tile
from concourse import bacc, mybir
from concourse.bass_utils import run_bass_kernel_spmd

N = 20000
E = 80000
D = 1024
C = 8
NPC = N // C      # 2500 nodes per core
NT = 20           # node tiles per core
NP = NT * 128     # 2560 padded node slots
F32 = mybir.dt.float32
BF16 = mybir.dt.bfloat16
I32 = mybir.dt.int32
BF = ml_dtypes.bfloat16

AF = mybir.ActivationFunctionType

_PROGRAM_CACHE = {}
DEFAULT_PROGRAM_ARGS = (80, 63, (4,) * 20)


def _build_program(NG, GU, pattern):
    """NG: total 128-edge groups/core (sum of pattern); GU: 128-row groups of
    unique source nodes; pattern: groups per node tile (len NT)."""
    assert sum(pattern) == NG and len(pattern) == NT
    UP = GU * 128
    S = NG * 128
    KC = D // 128  # 8

    nc = bacc.Bacc("TRN2", target_bir_lowering=False, debug=False, num_devices=C)

    # ---- DRAM I/O ----
    xtu_d = nc.dram_tensor("xtu_d", [D, UP], BF16, kind="ExternalInput").ap()
    eat_d = nc.dram_tensor("eat_d", [D, S], BF16, kind="ExternalInput").ap()
    sp_d = nc.dram_tensor("sp_d", [128, S], BF16, kind="ExternalInput").ap()
    srcidx = nc.dram_tensor("srcidx", [128, NG], I32, kind="ExternalInput").ap()
    x_myT = nc.dram_tensor("x_myT", [D, NP], BF16, kind="ExternalInput").ap()
    maskv = nc.dram_tensor("maskv", [1, NP], BF16, kind="ExternalInput").ap()
    ones_d = nc.dram_tensor("ones_d", [1, 128], BF16, kind="ExternalInput").ap()
    w1a_d = nc.dram_tensor("w1a_d", [2 * D, D], BF16, kind="ExternalInput").ap()
    wb1_d = nc.dram_tensor("wb1_d", [D, D], BF16, kind="ExternalInput").ap()
    w3_d = nc.dram_tensor("w3_d", [D, D], BF16, kind="ExternalInput").ap()
    w2b_d = nc.dram_tensor("w2b_d", [D, D], BF16, kind="ExternalInput").ap()
    b1a_d = nc.dram_tensor("b1a_d", [1, D], BF16, kind="ExternalInput").ap()
    u_d = nc.dram_tensor("u_d", [1, D], BF16, kind="ExternalInput").ap()
    b2a_d = nc.dram_tensor("b2a_d", [128, 8], F32, kind="ExternalInput").ap()
    b2b_d = nc.dram_tensor("b2b_d", [128, 8], F32, kind="ExternalInput").ap()
    out_myT = nc.dram_tensor("out_myT", [D, NP], BF16, kind="ExternalOutput").ap()
    xw_dram = nc.dram_tensor("xw_scratch", [UP, D], BF16).ap()

    NSB_U = (GU + 3) // 4   # xTu superblocks of 512 cols
    NSB_E = (NG + 3) // 4   # eaT/sp superblocks of 512 cols

    with tile.TileContext(nc) as tc, ExitStack() as ctx:
        cpool = ctx.enter_context(tc.tile_pool(name="consts", bufs=1))
        wpool = ctx.enter_context(tc.tile_pool(name="weights", bufs=1))
        sxu = ctx.enter_context(tc.tile_pool(name="sxu", bufs=2))
        sea = ctx.enter_context(tc.tile_pool(name="sea", bufs=2))
        ssp = ctx.enter_context(tc.tile_pool(name="ssp", bufs=2))
        pgat = ctx.enter_context(tc.tile_pool(name="pgat", bufs=6))
        pg = ctx.enter_context(tc.tile_pool(name="pg", bufs=3))
        pxw = ctx.enter_context(tc.tile_pool(name="pxw", bufs=3))
        prm = ctx.enter_context(tc.tile_pool(name="prm", bufs=2))
        pxt = ctx.enter_context(tc.tile_pool(name="pxt", bufs=2))
        po1 = ctx.enter_context(tc.tile_pool(name="po1", bufs=2))
        psBig = ctx.enter_context(tc.tile_pool(name="psBig", bufs=2, space="PSUM"))
        psPr = ctx.enter_context(tc.tile_pool(name="psPr", bufs=1, space="PSUM"))
        psMm2 = ctx.enter_context(tc.tile_pool(name="psMm2", bufs=2, space="PSUM"))

        # ---- phase-1-critical loads first (PE can start after ~3 MB) ----
        ones_sb = cpool.tile([1, 128], BF16, tag="ones")
        nc.sync.dma_start(ones_sb[:], ones_d[:])
        b1a_sb = cpool.tile([1, D], BF16, tag="b1a")
        nc.sync.dma_start(b1a_sb[:], b1a_d[:])
        w1a_sb = []
        xu = []
        hi0 = min(512, UP)
        for k in range(KC):
            tx = sxu.tile([128, 512], BF16, tag=f"xu{k}", name=f"xu0_{k}")
            nc.sync.dma_start(tx[:, :hi0], xtu_d[128 * k : 128 * (k + 1), :hi0])
            xu.append(tx)
            t = wpool.tile([128, D], BF16, tag=f"w1a{k}")
            nc.sync.dma_start(t[:], w1a_d[128 * k : 128 * (k + 1), :])
            w1a_sb.append(t)

        # ================= Phase 1: xw = x_u @ W1a_top + b1a =================
        for ug in range(GU):
            sb, col = ug // 4, ug % 4
            if col == 0 and sb > 0:
                xu = []
                hi = min(512, UP - 512 * sb)
                for k in range(KC):
                    t = sxu.tile([128, 512], BF16, tag=f"xu{k}", name=f"xu{sb}_{k}")
                    nc.sync.dma_start(
                        t[:, :hi],
                        xtu_d[128 * k : 128 * (k + 1), 512 * sb : 512 * sb + hi],
                    )
                    xu.append(t)
            pw = psBig.tile([128, D], F32, tag="big", name=f"pw{ug}")
            for h in range(2):
                for k in range(KC):
                    nc.tensor.matmul(
                        pw[:, 512 * h : 512 * (h + 1)],
                        xu[k][:, 128 * col : 128 * (col + 1)],
                        w1a_sb[k][:, 512 * h : 512 * (h + 1)],
                        start=(k == 0),
                        stop=False,
                    )
                nc.tensor.matmul(
                    pw[:, 512 * h : 512 * (h + 1)],
                    ones_sb[:],
                    b1a_sb[0:1, 512 * h : 512 * (h + 1)],
                    start=False,
                    stop=True,
                )
            xw_sb = pxw.tile([128, D], BF16, tag="xw", name=f"xwsb{ug}")
            nc.vector.tensor_copy(xw_sb[:], pw[:])
            nc.sync.dma_start(xw_dram[128 * ug : 128 * (ug + 1), :], xw_sb[:])

        # ---- remaining constants + phase 2/3 weights (overlap phase 1) ----
        srcidx_sb = cpool.tile([128, NG], I32, tag="srcidx")
        nc.sync.dma_start(srcidx_sb[:], srcidx[:])
        u_sb = cpool.tile([1, D], BF16, tag="u")
        nc.sync.dma_start(u_sb[:], u_d[:])
        b2a_sb = cpool.tile([128, 8], F32, tag="b2a")
        nc.sync.dma_start(b2a_sb[:], b2a_d[:])
        b2b_sb = cpool.tile([128, 8], F32, tag="b2b")
        nc.sync.dma_start(b2b_sb[:], b2b_d[:])
        for k in range(KC, 2 * KC):
            t = wpool.tile([128, D], BF16, tag=f"w1a{k}")
            nc.sync.dma_start(t[:], w1a_d[128 * k : 128 * (k + 1), :])
            w1a_sb.append(t)
        wb1_sb, w3_sb, w2b_sb = [], [], []
        for wd, lst, nm in ((wb1_d, wb1_sb, "wb"), (w3_d, w3_sb, "w3"),
                            (w2b_d, w2b_sb, "w2")):
            for k in range(KC):
                t = wpool.tile([128, D], BF16, tag=f"{nm}{k}")
                nc.sync.dma_start(t[:], wd[128 * k : 128 * (k + 1), :])
                lst.append(t)

        # ============ Phase 2: fused edge MLP1 + segment means ============
        # flattened group schedule; S-matmuls for group i are emitted after
        # group i+1's MLP1 matmuls so the relu (ACT) overlaps PE work, and the
        # identity-matmul (gathered xw) comes last so the gather has the whole
        # MLP1 window to land.
        items = []
        jj = 0
        for q in range(NT):
            for j in range(pattern[q]):
                items.append((q, j, jj))
                jj += 1

        state = {}   # jj -> (g_sb, spt, col, prt, first, last, q)
        rmt_by_t2 = {}
        prt_by_q = {}
        eat = spt = None

        def finalize(idx):
            """Emit deferred S-matmuls for group idx (+ tile epilogue)."""
            g_sb, spt_, col_, prt_, first, last, q_ = state.pop(idx)
            # prt spans 2 PSUM banks (4 chunks each); start=True clears
            # has_written for the WHOLE bank, so only the first chunk per
            # bank may set it — the others overwrite-on-cleared-bit.
            for k in range(KC):
                nc.tensor.matmul(
                    prt_[:, 128 * k : 128 * (k + 1)],
                    g_sb[:, 128 * k : 128 * (k + 1)],
                    spt_[:, 128 * col_ : 128 * (col_ + 1)],
                    start=(first and k % 4 == 0),
                    stop=last,
                )
            if last:
                t2_, half_ = q_ // 2, q_ % 2
                rmt_ = rmt_by_t2[t2_]
                for k in range(KC):
                    nc.vector.tensor_copy(
                        rmt_[k][:, 128 * half_ : 128 * (half_ + 1)],
                        prt_[:, 128 * k : 128 * (k + 1)],
                    )
                if half_ == 1:
                    mlp2(t2_, rmt_)

        xt_by_t2 = {}

        def prefetch_xt(t2):
            xt = []
            for k in range(KC):
                t = pxt.tile([128, 256], BF16, tag=f"xt{k}", name=f"xt{t2}_{k}")
                nc.sync.dma_start(
                    t[:],
                    x_myT[128 * k : 128 * (k + 1), 256 * t2 : 256 * (t2 + 1)],
                )
                xt.append(t)
            msk = pxt.tile([1, 256], BF16, tag="msk", name=f"msk{t2}")
            nc.sync.dma_start(msk[:], maskv[0:1, 256 * t2 : 256 * (t2 + 1)])
            xt_by_t2[t2] = (xt, msk)

        def mlp2(t2, rmt):
            xt, msk = xt_by_t2.pop(t2)

            o1 = []
            for m in range(KC):
                pb = psMm2.tile([128, 256], F32, tag="pb", name=f"pa{t2}_{m}")
                for k in range(KC):
                    nc.tensor.matmul(
                        pb[:], wb1_sb[k][:, 128 * m : 128 * (m + 1)], xt[k][:],
                        start=(k == 0), stop=False,
                    )
                # W3 split: the even tile's rm half (cols 0:128) was copied a
                # whole tile earlier; the odd half just landed, so do it last
                for k in range(KC):
                    nc.tensor.matmul(
                        pb[:, 0:128],
                        w3_sb[k][:, 128 * m : 128 * (m + 1)],
                        rmt[k][:, 0:128],
                        start=False, stop=False,
                    )
                nc.tensor.matmul(
                    pb[:], u_sb[0:1, 128 * m : 128 * (m + 1)], msk[:],
                    start=False, stop=False,
                )
                for k in range(KC):
                    nc.tensor.matmul(
                        pb[:, 128:256],
                        w3_sb[k][:, 128 * m : 128 * (m + 1)],
                        rmt[k][:, 128:256],
                        start=False, stop=(k == KC - 1),
                    )
                ot = po1.tile([128, 256], BF16, tag=f"o1{m}", name=f"o1{t2}_{m}")
                nc.scalar.activation(ot[:], pb[:], AF.Relu,
                                     bias=b2a_sb[:, m : m + 1])
                o1.append(ot)

            for m in range(KC):
                pb = psMm2.tile([128, 256], F32, tag="pb", name=f"pb{t2}_{m}")
                for k in range(KC):
                    nc.tensor.matmul(
                        pb[:], w2b_sb[k][:, 128 * m : 128 * (m + 1)], o1[k][:],
                        start=(k == 0), stop=(k == KC - 1),
                    )
                ot = po1.tile([128, 256], BF16, tag="o2", name=f"o2{t2}_{m}",
                              bufs=3)
                nc.scalar.activation(ot[:], pb[:], AF.Identity,
                                     bias=b2b_sb[:, m : m + 1])
                nc.sync.dma_start(
                    out_myT[128 * m : 128 * (m + 1), 256 * t2 : 256 * (t2 + 1)],
                    ot[:],
                )

        for q, j, jj in items:
            t2, half = q // 2, q % 2
            if half == 0 and j == 0:
                rmt_by_t2[t2] = [
                    prm.tile([128, 256], BF16, tag=f"rm{k}", name=f"rm{t2}_{k}")
                    for k in range(KC)
                ]
                prefetch_xt(t2)
            if j == 0:
                prt_by_q[q] = psPr.tile([128, D], F32, tag="pr", name=f"prt{q}")
            prt = prt_by_q[q]
            Gq = pattern[q]

            sb, col = jj // 4, jj % 4
            if col == 0:
                eat = []
                hi = min(512, S - 512 * sb)
                for k in range(KC):
                    t = sea.tile([128, 512], BF16, tag=f"ea{k}",
                                 name=f"ea{sb}_{k}")
                    nc.sync.dma_start(
                        t[:, :hi],
                        eat_d[128 * k : 128 * (k + 1), 512 * sb : 512 * sb + hi],
                    )
                    eat.append(t)
                spt = ssp.tile([128, 512], BF16, tag="sp", name=f"sp{sb}")
                nc.sync.dma_start(spt[:, :hi], sp_d[:, 512 * sb : 512 * sb + hi])

            xwg = pgat.tile([128, D], BF16, tag="xwg", name=f"xwg{jj}")
            nc.gpsimd.indirect_dma_start(
                out=xwg[:],
                out_offset=None,
                in_=xw_dram[:],
                in_offset=bass.IndirectOffsetOnAxis(
                    ap=srcidx_sb[:, jj : jj + 1], axis=0
                ),
            )

            ph = psBig.tile([128, D], F32, tag="big", name=f"ph{jj}")
            for h in range(2):
                for k in range(KC):
                    nc.tensor.matmul(
                        ph[:, 512 * h : 512 * (h + 1)],
                        eat[k][:, 128 * col : 128 * (col + 1)],
                        w1a_sb[KC + k][:, 512 * h : 512 * (h + 1)],
                        start=(k == 0),
                        stop=(k == KC - 1),
                    )
            if jj > 0:
                finalize(jj - 1)
            gt = pg.tile([128, D], BF16, tag="gt", name=f"gt{jj}")
            nc.vector.tensor_tensor(out=gt[:], in0=ph[:], in1=xwg[:],
                                    op=mybir.AluOpType.add)
            g_sb = pg.tile([128, D], BF16, tag="g", name=f"g{jj}")
            nc.scalar.activation(g_sb[:], gt[:], AF.Relu)
            state[jj] = (g_sb, spt, col, prt, j == 0, j == Gq - 1, q)
        finalize(NG - 1)

    nc.compile()
    return nc


def _get_program(NG, GU, pattern):
    key = (NG, GU, tuple(pattern))
    if key not in _PROGRAM_CACHE:
        _PROGRAM_CACHE[key] = _build_program(NG, GU, tuple(pattern))
    return _PROGRAM_CACHE[key]


def _pack_core(deg):
    """Pack NPC nodes (weights deg) into NT bins, <=128 nodes each,
    minimizing sum(ceil(load/128)). Returns list of (nodes, load)."""
    order = np.argsort(-deg, kind="stable")
    nodes = [[] for _ in range(NT)]
    load = np.zeros(NT, np.int64)
    cnt = np.zeros(NT, np.int64)
    for n in order:
        # LPT with node cap
        cand = [b for b in range(NT) if cnt[b] < 128]
        b = min(cand, key=lambda b: (load[b], cnt[b]))
        nodes[b].append(n)
        load[b] += deg[n]
        cnt[b] += 1
    # refinement: reduce sum(ceil(load/128)) by moving small nodes out of
    # bins that spill just over a multiple of 128
    for _ in range(200):
        ceil = -(-load // 128)
        improved = False
        spill_key = np.where(
            (load > 0) & (load % 128 != 0), (load - 1) % 128 + 1, 10**9
        )
        for a in np.argsort(spill_key):
            if load[a] == 0 or (load[a] % 128) == 0:
                continue
            spill = load[a] - 128 * (ceil[a] - 1)
            # try to move small nodes (total <= spill) from a to other bins
            small = sorted((deg[n], n) for n in nodes[a] if deg[n] > 0)
            moved = []
            need = spill
            for d, n in small:
                if d > need:
                    break
                tgt = None
                for b in range(NT):
                    if b == a or cnt[b] >= 128:
                        continue
                    if -(-(load[b] + d) // 128) == ceil[b]:
                        tgt = b
                        break
                if tgt is None:
                    continue
                nodes[a].remove(n)
                nodes[tgt].append(n)
                load[a] -= d
                load[tgt] += d
                cnt[a] -= 1
                cnt[tgt] += 1
                moved.append(n)
                need -= d
                if need <= 0:
                    break
            if need <= 0 and moved:
                improved = True
                break
        if not improved:
            break
    return [(nodes[b], int(load[b])) for b in range(NT)]


def _make_in_maps(x, edge_index, edge_attr, W1a, b1a, W1b, b1b, W2a, b2a, W2b, b2b):
    x = np.ascontiguousarray(np.asarray(x, np.float32))
    edge_attr = np.ascontiguousarray(np.asarray(edge_attr, np.float32))
    ei = np.asarray(edge_index)
    row, col = ei[0].astype(np.int64), ei[1].astype(np.int64)

    perm = np.argsort(col, kind="stable")
    col_s, row_s = col[perm], row[perm]
    core_bounds = np.searchsorted(col_s, NPC * np.arange(C + 1))
    counts = np.bincount(col, minlength=N)

    # ---- pack nodes per core; derive the shared group pattern ----
    packs, uniqs = [], []
    for c in range(C):
        deg = counts[NPC * c : NPC * (c + 1)]
        bins = _pack_core(deg)
        bins.sort(key=lambda bl: -bl[1])
        packs.append(bins)
        s0, e0 = core_bounds[c], core_bounds[c + 1]
        uniqs.append(np.unique(row_s[s0:e0]))
    pattern = tuple(
        int(max(-(-packs[c][q][1] // 128) for c in range(C))) for q in range(NT)
    )
    pattern = tuple(max(p, 1) for p in pattern)
    NG = sum(pattern)
    GU = max(1, -(-max(len(u) for u in uniqs) // 128))
    UP, S = GU * 128, NG * 128

    # ---- fold weights (float64 for accuracy) ----
    B1 = np.asarray(W2a, np.float64)[:D]
    B2 = np.asarray(W2a, np.float64)[D:]
    W3 = (np.asarray(W1b, np.float64) @ B2).astype(np.float32)
    u_vec = (np.asarray(b1b, np.float64) @ B2).astype(np.float32)

    x_bf = x.astype(BF)
    ea_bf = edge_attr.astype(BF)

    in_maps = []
    orders = []
    for c in range(C):
        s0 = core_bounds[c]
        lo = NPC * c
        bins = packs[c]
        uniq = uniqs[c]
        deg = counts[lo : lo + NPC]
        starts = np.zeros(NPC + 1, np.int64)
        np.cumsum(deg, out=starts[1:])

        src_l = np.zeros((128, NG), np.int32)     # local xw row per slot
        sp = np.zeros((128, S), BF)               # selection matrix, 1/deg vals
        ea_sel = np.full(S, -1, np.int64)         # edge_attr row per slot
        order = np.full(NP, -1, np.int64)         # packed node order

        goff = 0
        for q in range(NT):
            bnodes, load = bins[q]
            Gq = pattern[q]
            pos = 0
            for p, n in enumerate(bnodes):
                order[128 * q + p] = n
                d = int(deg[n])
                if d == 0:
                    continue
                ids = np.arange(starts[n], starts[n + 1], dtype=np.int64)
                sl = goff * 128 + pos + np.arange(d)
                gidx_, ridx_ = sl // 128, sl % 128
                src_l[ridx_, gidx_] = np.searchsorted(uniq, row_s[s0 + ids])
                sp[ridx_, gidx_ * 128 + p] = BF(1.0 / d)
                ea_sel[sl] = perm[s0 + ids]
                pos += d
            assert pos <= 128 * Gq, (c, q, pos, Gq)
            goff += Gq
        assert goff == NG

        # host-transposed, slot-ordered edge features
        eat = np.zeros((D, S), BF)
        nz = ea_sel >= 0
        eat[:, nz] = ea_bf[ea_sel[nz]].T
        # host-transposed unique-source features
        xtu = np.zeros((D, UP), BF)
        xtu[:, : len(uniq)] = x_bf[uniq].T

        ordc = np.maximum(order, 0)
        valid = order >= 0
        cnt_c = np.where(valid, deg[ordc], 0)
        mask_c = ((cnt_c > 0) & valid).astype(BF)
        x_c = np.where(valid[:, None], x_bf[lo + ordc], BF(0.0))

        in_maps.append(
            {
                "xtu_d": xtu,
                "eat_d": eat,
                "sp_d": sp,
                "srcidx": src_l,
                "x_myT": np.ascontiguousarray(x_c.T),
                "maskv": mask_c.reshape(1, NP),
                "ones_d": np.ones((1, 128), BF),
                "w1a_d": np.asarray(W1a, np.float32).astype(BF),
                "wb1_d": B1.astype(np.float32).astype(BF),
                "w3_d": W3.astype(BF),
                "w2b_d": np.asarray(W2b, np.float32).astype(BF),
                "b1a_d": np.asarray(b1a, np.float32).astype(BF).reshape(1, D),
                "u_d": u_vec.astype(BF).reshape(1, D),
                "b2a_d": np.asarray(b2a, np.float32).reshape(8, 128).T.copy(),
                "b2b_d": np.asarray(b2b, np.float32).reshape(8, 128).T.copy(),
            }
        )
        orders.append(order)
    return (NG, GU, pattern), in_maps, orders


def kernel(x, edge_index, edge_attr, W1a, b1a, W1b, b1b, W2a, b2a, W2b, b2b):
    args, in_maps, orders = _make_in_maps(
        x, edge_index, edge_attr, W1a, b1a, W1b, b1b, W2a, b2a, W2b, b2b
    )
    nc = _get_program(*args)
    res = run_bass_kernel_spmd(nc, in_maps, core_ids=list(range(C)))
    out = np.empty((N, D), np.float32)
    for c in range(C):
        o = np.asarray(res.results[c]["out_myT"]).astype(np.float32).T  # [NP, D]
        order = orders[c]
        valid = order >= 0
        out[NPC * c + order[valid]] = o[valid]
    return np.ascontiguousarray(out)
